# revision 23
# baseline (speedup 1.0000x reference)
"""VQ codebook squared-distance kernel for Trainium2 (8 NeuronCores).

Computes dist[n,k,l] = (||x[n,:,l]||^2 + ||w[k,:]||^2 - 2*x[n,:,l].w[k,:]) / scale^2
for x (32,128,3136) f32, weight (64,128) f32, scale (1,) f32 -> out (32,64,3136) f32.

Sharding: data-parallel over N (4 per core); weight/scale replicated.

Per-core design (fp16 PE path):
  - inputs: fp32 halves on the sync HWDGE ring (dedicated trigger FIFO);
    outputs: fp32 quarters via gpsimd SWDGE (separate trigger engine, so
    output sem-waits never block input triggers).
  - DVE casts x -> fp16 (2x_2P mode); ACT computes x^2 fp32->fp16 (Square).
  - PE: psum = (-2Wt)f16 @ x_f16 + ones_f16 @ (x^2)_f16, two n's per PSUM
    tile via column tiling; psum tiles span 2 banks (2x 512 cols) so one
    DVE epilogue covers 1024 columns: out = (psum + ||c_k||^2) / scale^2.
"""

import numpy as np

N, D, L, K = 32, 128, 3136, 64
N_CORES = 8
NS = N // N_CORES          # n's per core
LC = 512                   # matmul chunk (one PSUM bank)
LH = L // 2                # half length for input DMA / cast / square
LQ = L // 4                # quarter length for output DMA

_cache = {}


def _build():
    import concourse.bacc as bacc
    import concourse.mybir as mybir
    import concourse.tile as tile
    from concourse.masks import make_identity

    f32 = mybir.dt.float32
    f16 = mybir.dt.float16
    AF = mybir.ActivationFunctionType

    nc = bacc.Bacc(
        "TRN2",
        target_bir_lowering=False,
        debug=False,
        enable_asserts=False,
        num_devices=N_CORES,
    )

    x_ap = nc.dram_tensor("x", (NS, D, L), f32, kind="ExternalInput").ap()
    w_ap = nc.dram_tensor("weight", (K, D), f32, kind="ExternalInput").ap()
    s_ap = nc.dram_tensor("scale", (1,), f32, kind="ExternalInput").ap()
    o_ap = nc.dram_tensor("out", (NS, K, L), f32, kind="ExternalOutput").ap()

    with tile.TileContext(nc) as tc:
        with (
            tc.tile_pool(name="consts", bufs=1) as consts,
            tc.tile_pool(name="xin", bufs=4) as xpool,
            tc.tile_pool(name="xsq", bufs=3) as xqpool,
            tc.tile_pool(name="outp", bufs=2) as opool,
            tc.tile_pool(name="psum", bufs=3, space="PSUM") as pspool,
            tc.tile_pool(name="psum1", bufs=1, space="PSUM") as pspool1,
        ):
            # ---- input loads first: earliest possible trigger issue.
            # Chain load i after completion of load i-2 (2-deep window) so
            # halves arrive in consumption order instead of SDMA round-robin
            # finishing everything late together.
            from concourse.tile_rust import add_dep_helper

            rings = [nc.sync, nc.scalar]
            xfs = []
            in_dmas = []
            for n in range(NS):
                xf = xpool.tile([D, L], f32, tag="xf", name=f"xf_{n}")
                for h in range(2):
                    hs = slice(h * LH, (h + 1) * LH)
                    dma = rings[len(in_dmas) % 2].dma_start(
                        out=xf[:, hs], in_=x_ap[n][:, hs]
                    )
                    if len(in_dmas) >= 2:
                        add_dep_helper(dma.ins, in_dmas[-2].ins, reason="in-order input stream")
                    in_dmas.append(dma)
                xfs.append(xf)

            # ---- constants -------------------------------------------------
            w2 = consts.tile([2 * K, D], f32)
            nc.sync.dma_start(out=w2[0:K, :], in_=w_ap)
            nc.sync.dma_start(out=w2[K : 2 * K, :], in_=w_ap)

            s_b = consts.tile([128, 1], f32)
            nc.gpsimd.dma_start(out=s_b, in_=s_ap.to_broadcast((128, 1)))
            inv_s2 = consts.tile([128, 1], f32)
            nc.vector.tensor_mul(inv_s2, s_b, s_b)
            nc.vector.reciprocal(inv_s2, inv_s2)

            ident = consts.tile([K, K], f32)
            make_identity(nc, ident)
            ps_w = pspool1.tile([D, K], f32)
            nc.tensor.transpose(ps_w, w2[0:K, :], ident)
            wT16 = consts.tile([D, K], f16)
            nc.vector.tensor_scalar_mul(wT16, in0=ps_w, scalar1=-2.0)

            ones16 = consts.tile([D, K], f16)
            nc.vector.memset(ones16, 1.0)

            w_sq = consts.tile([2 * K, D], f32)
            nc.vector.tensor_mul(w_sq, w2, w2)
            c_sq = consts.tile([2 * K, 1], f32)
            nc.vector.reduce_sum(out=c_sq, in_=w_sq, axis=mybir.AxisListType.X)

            # ---- main loop -------------------------------------------------
            for pair in range(NS // 2):
                xs = []
                for s in range(2):
                    n = 2 * pair + s
                    xf = xfs[n]
                    xt = xpool.tile([D, L], f16, tag="xt", name=f"x_{n}")
                    xq = xqpool.tile([D, L], f16, tag="xq", name=f"xsq_{n}")
                    for h in range(2):
                        hs = slice(h * LH, (h + 1) * LH)
                        nc.vector.tensor_copy(out=xt[:, hs], in_=xf[:, hs])
                        nc.scalar.activation(xq[:, hs], xf[:, hs], AF.Square)
                    xs.append((xt, xq))

                out_t = opool.tile([2 * K, L], f32, tag="out_t", name=f"out_{pair}")
                # 3 double-bank chunks of 2x512, then a 64-col tail
                spans = [(0, 1024), (1024, 2048), (2048, 3072), (3072, 3136)]
                for c0, c1 in spans:
                    ps = pspool.tile([2 * K, 1024], f32, name="ps")[:, : c1 - c0]
                    for b0 in range(0, c1 - c0, LC):
                        b1 = min(b0 + LC, c1 - c0)
                        pb = ps[:, b0:b1]
                        sl = slice(c0 + b0, c0 + b1)
                        nc.tensor.matmul(
                            pb[0:K, :], wT16, xs[0][0][:, sl],
                            start=True, stop=False, tile_position=(0, 0),
                        )
                        nc.tensor.matmul(
                            pb[K : 2 * K, :], wT16, xs[1][0][:, sl],
                            start=True, stop=False, tile_position=(0, 64),
                        )
                        nc.tensor.matmul(
                            pb[0:K, :], ones16, xs[0][1][:, sl],
                            start=False, stop=True, tile_position=(0, 0),
                        )
                        nc.tensor.matmul(
                            pb[K : 2 * K, :], ones16, xs[1][1][:, sl],
                            start=False, stop=True, tile_position=(0, 64),
                        )
                    nc.vector.tensor_scalar(
                        out=out_t[:, c0:c1], in0=ps,
                        scalar1=c_sq, scalar2=inv_s2,
                        op0=mybir.AluOpType.add, op1=mybir.AluOpType.mult,
                    )
                o_pair = o_ap[2 * pair : 2 * pair + 2].rearrange("a k l -> (a k) l")
                for q in range(4):
                    qs = slice(q * LQ, (q + 1) * LQ)
                    nc.gpsimd.dma_start(out=o_pair[:, qs], in_=out_t[:, qs])

    nc.compile()
    return nc


def _get_nc():
    if "nc" not in _cache:
        _cache["nc"] = _build()
    return _cache["nc"]


def run(x, weight, scale, trace=False):
    from concourse.bass_utils import run_bass_kernel_spmd

    x = np.ascontiguousarray(np.asarray(x, dtype=np.float32))
    weight = np.ascontiguousarray(np.asarray(weight, dtype=np.float32))
    scale = np.ascontiguousarray(np.asarray(scale, dtype=np.float32))
    assert x.shape == (N, D, L) and weight.shape == (K, D) and scale.shape == (1,)

    nc = _get_nc()
    in_maps = [
        {"x": x[c * NS : (c + 1) * NS], "weight": weight, "scale": scale}
        for c in range(N_CORES)
    ]
    res = run_bass_kernel_spmd(
        nc, in_maps, core_ids=list(range(N_CORES)), trace=trace
    )
    out = np.concatenate([r["out"] for r in res.results], axis=0)
    return out, res


def kernel(x, weight, scale):
    out, _ = run(x, weight, scale, trace=False)
    return out


# revision 24
# speedup vs baseline: 1.0986x; 1.0986x over previous
"""VQ codebook squared-distance kernel for Trainium2 (8 NeuronCores).

Computes dist[n,k,l] = (||x[n,:,l]||^2 + ||w[k,:]||^2 - 2*x[n,:,l].w[k,:]) / scale^2
for x (32,128,3136) f32, weight (64,128) f32, scale (1,) f32 -> out (32,64,3136) f32.

Sharding: data-parallel over N (4 per core); weight/scale replicated.
The kernel is HBM-bound: 9.64 MB/core over a stack shared with the paired
core caps at ~310 GB/s, so the structure exists to keep the DMA stream
saturated; all compute hides under it.

Per-core design (fp16 PE path):
  - inputs: 8 fp32 half-tiles, all on the sync HWDGE ring (clean trigger
    FIFO); outputs on the scalar ring (disjoint trigger FIFO).
  - DVE casts x -> fp16 (2x_2P); ACT computes x^2 -> fp16 (Square, fp32 in).
  - PE: psum = (-2Wt)f16 @ x_f16 + ones_f16 @ (x^2)_f16, two n's per PSUM
    tile via column tiling (tile_position (0,0)/(0,64)); psum tiles span
    2 banks so one DVE epilogue covers 1024 cols:
    out = (psum + ||c_k||^2) / scale^2.
"""

import numpy as np

N, D, L, K = 32, 128, 3136, 64
N_CORES = 8
NS = N // N_CORES          # n's per core
LC = 512                   # matmul chunk (one PSUM bank)
LH = L // 2                # half length for input DMA

_cache = {}


def _build():
    import concourse.bacc as bacc
    import concourse.mybir as mybir
    import concourse.tile as tile
    from concourse.masks import make_identity

    f32 = mybir.dt.float32
    f16 = mybir.dt.float16
    AF = mybir.ActivationFunctionType

    nc = bacc.Bacc(
        "TRN2",
        target_bir_lowering=False,
        debug=False,
        enable_asserts=False,
        num_devices=N_CORES,
    )

    x_ap = nc.dram_tensor("x", (NS, D, L), f32, kind="ExternalInput").ap()
    w_ap = nc.dram_tensor("weight", (K, D), f32, kind="ExternalInput").ap()
    s_ap = nc.dram_tensor("scale", (1,), f32, kind="ExternalInput").ap()
    o_ap = nc.dram_tensor("out", (NS, K, L), f32, kind="ExternalOutput").ap()

    with tile.TileContext(nc) as tc:
        with (
            tc.tile_pool(name="consts", bufs=1) as consts,
            tc.tile_pool(name="xin", bufs=4) as xpool,
            tc.tile_pool(name="xsq", bufs=4) as xqpool,
            tc.tile_pool(name="outp", bufs=2) as opool,
            tc.tile_pool(name="psum", bufs=3, space="PSUM") as pspool,
            tc.tile_pool(name="psum1", bufs=1, space="PSUM") as pspool1,
        ):
            # ---- input stream: all x halves on the sync ring, in order ----
            xfs = []
            for n in range(NS):
                xf = xpool.tile([D, L], f32, tag="xf", name=f"xf_{n}")
                for h in range(2):
                    hs = slice(h * LH, (h + 1) * LH)
                    nc.sync.dma_start(out=xf[:, hs], in_=x_ap[n][:, hs])
                xfs.append(xf)

            # ---- constants -------------------------------------------------
            w2 = consts.tile([2 * K, D], f32)
            nc.sync.dma_start(out=w2[0:K, :], in_=w_ap)
            nc.sync.dma_start(out=w2[K : 2 * K, :], in_=w_ap)

            s_b = consts.tile([128, 1], f32)
            nc.gpsimd.dma_start(out=s_b, in_=s_ap.to_broadcast((128, 1)))
            inv_s2 = consts.tile([128, 1], f32)
            nc.vector.tensor_mul(inv_s2, s_b, s_b)
            nc.vector.reciprocal(inv_s2, inv_s2)

            ident = consts.tile([K, K], f32)
            make_identity(nc, ident)
            ps_w = pspool1.tile([D, K], f32)
            nc.tensor.transpose(ps_w, w2[0:K, :], ident)
            wT16 = consts.tile([D, K], f16)
            nc.vector.tensor_scalar_mul(wT16, in0=ps_w, scalar1=-2.0)

            ones16 = consts.tile([D, K], f16)
            nc.vector.memset(ones16, 1.0)

            w_sq = consts.tile([2 * K, D], f32)
            nc.vector.tensor_mul(w_sq, w2, w2)
            c_sq = consts.tile([2 * K, 1], f32)
            nc.vector.reduce_sum(out=c_sq, in_=w_sq, axis=mybir.AxisListType.X)

            # ---- derived streams: fp16 x and fp16 x^2 per n ---------------
            xts, xqs = [], []
            for n in range(NS):
                xt = xpool.tile([D, L], f16, tag="xt", name=f"x_{n}")
                nc.vector.tensor_copy(out=xt, in_=xfs[n])
                xq = xqpool.tile([D, L], f16, tag="xq", name=f"xsq_{n}")
                nc.scalar.activation(xq, xfs[n], AF.Square)
                xts.append(xt)
                xqs.append(xq)

            # ---- matmuls + epilogue per pair ------------------------------
            spans = [(0, 1024), (1024, 2048), (2048, 3072), (3072, 3136)]
            out_ts = []
            for pair in range(NS // 2):
                n0, n1 = 2 * pair, 2 * pair + 1
                out_t = opool.tile([2 * K, L], f32, tag="out_t", name=f"out_{pair}")
                out_ts.append(out_t)
                for c0, c1 in spans:
                    ps = pspool.tile([2 * K, 1024], f32, name="ps")[:, : c1 - c0]
                    for b0 in range(0, c1 - c0, LC):
                        b1 = min(b0 + LC, c1 - c0)
                        pb = ps[:, b0:b1]
                        sl = slice(c0 + b0, c0 + b1)
                        nc.tensor.matmul(
                            pb[0:K, :], wT16, xts[n0][:, sl],
                            start=True, stop=False, tile_position=(0, 0),
                        )
                        nc.tensor.matmul(
                            pb[K : 2 * K, :], wT16, xts[n1][:, sl],
                            start=True, stop=False, tile_position=(0, 64),
                        )
                        nc.tensor.matmul(
                            pb[0:K, :], ones16, xqs[n0][:, sl],
                            start=False, stop=True, tile_position=(0, 0),
                        )
                        nc.tensor.matmul(
                            pb[K : 2 * K, :], ones16, xqs[n1][:, sl],
                            start=False, stop=True, tile_position=(0, 64),
                        )
                    nc.vector.tensor_scalar(
                        out=out_t[:, c0:c1], in0=ps,
                        scalar1=c_sq, scalar2=inv_s2,
                        op0=mybir.AluOpType.add, op1=mybir.AluOpType.mult,
                    )

            # ---- output stores on the scalar ring (tapered tail) ----------
            o0 = o_ap[0:2].rearrange("a k l -> (a k) l")
            nc.scalar.dma_start(out=o0, in_=out_ts[0])
            o1 = o_ap[2:4].rearrange("a k l -> (a k) l")
            for h in range(2):
                hs = slice(h * LH, (h + 1) * LH)
                nc.scalar.dma_start(out=o1[:, hs], in_=out_ts[1][:, hs])

    nc.compile()
    return nc


def _get_nc():
    if "nc" not in _cache:
        _cache["nc"] = _build()
    return _cache["nc"]


def run(x, weight, scale, trace=False):
    from concourse.bass_utils import run_bass_kernel_spmd

    x = np.ascontiguousarray(np.asarray(x, dtype=np.float32))
    weight = np.ascontiguousarray(np.asarray(weight, dtype=np.float32))
    scale = np.ascontiguousarray(np.asarray(scale, dtype=np.float32))
    assert x.shape == (N, D, L) and weight.shape == (K, D) and scale.shape == (1,)

    nc = _get_nc()
    in_maps = [
        {"x": x[c * NS : (c + 1) * NS], "weight": weight, "scale": scale}
        for c in range(N_CORES)
    ]
    res = run_bass_kernel_spmd(
        nc, in_maps, core_ids=list(range(N_CORES)), trace=trace
    )
    out = np.concatenate([r["out"] for r in res.results], axis=0)
    return out, res


def kernel(x, weight, scale):
    out, _ = run(x, weight, scale, trace=False)
    return out


# revision 25
# speedup vs baseline: 1.1947x; 1.0875x over previous
"""VQ codebook squared-distance kernel for Trainium2 (8 NeuronCores).

Computes dist[n,k,l] = (||x[n,:,l]||^2 + ||w[k,:]||^2 - 2*x[n,:,l].w[k,:]) / scale^2
for x (32,128,3136) f32, weight (64,128) f32, scale (1,) f32 -> out (32,64,3136) f32.

Sharding: data-parallel over N (4 per core); weight/scale replicated.
The kernel is HBM-bound: 9.64 MB/core over a stack shared with the paired
core caps at ~310 GB/s, so the structure exists to keep the DMA stream
saturated; all compute hides under it.

Per-core design (fp16 PE path):
  - inputs: 8 fp32 half-tiles, all on the sync HWDGE ring (clean trigger
    FIFO); outputs on the scalar ring (disjoint trigger FIFO).
  - DVE casts x -> fp16 (2x_2P); ACT computes x^2 -> fp16 (Square, fp32 in).
  - PE: psum = (-2Wt)f16 @ x_f16 + ones_f16 @ (x^2)_f16, two n's per PSUM
    tile via column tiling (tile_position (0,0)/(0,64)); psum tiles span
    2 banks so one DVE epilogue covers 1024 cols:
    out = (psum + ||c_k||^2) / scale^2.
"""

import numpy as np

N, D, L, K = 32, 128, 3136, 64
N_CORES = 8
NS = N // N_CORES          # n's per core
LC = 512                   # matmul chunk (one PSUM bank)
LH = L // 2                # half length for input DMA

_cache = {}


def _build():
    import concourse.bacc as bacc
    import concourse.mybir as mybir
    import concourse.tile as tile
    from concourse.masks import make_identity

    f32 = mybir.dt.float32
    f16 = mybir.dt.float16
    AF = mybir.ActivationFunctionType

    nc = bacc.Bacc(
        "TRN2",
        target_bir_lowering=False,
        debug=False,
        enable_asserts=False,
        num_devices=N_CORES,
    )

    x_ap = nc.dram_tensor("x", (NS, D, L), f32, kind="ExternalInput").ap()
    w_ap = nc.dram_tensor("weight", (K, D), f32, kind="ExternalInput").ap()
    s_ap = nc.dram_tensor("scale", (1,), f32, kind="ExternalInput").ap()
    o_ap = nc.dram_tensor("out", (NS, K, L), f32, kind="ExternalOutput").ap()

    with tile.TileContext(nc) as tc:
        with (
            tc.tile_pool(name="consts", bufs=1) as consts,
            tc.tile_pool(name="xin", bufs=4) as xpool,
            tc.tile_pool(name="xsq", bufs=4) as xqpool,
            tc.tile_pool(name="outp", bufs=2) as opool,
            tc.tile_pool(name="psum", bufs=3, space="PSUM") as pspool,
            tc.tile_pool(name="psum1", bufs=1, space="PSUM") as pspool1,
        ):
            # ---- input stream: SWDGE cast-on-load fp32->fp16 halves --------
            xts = []
            for n in range(NS):
                xt = xpool.tile([D, L], f16, tag="xt", name=f"x_{n}")
                for h in range(2):
                    hs = slice(h * LH, (h + 1) * LH)
                    nc.gpsimd.dma_start(out=xt[:, hs], in_=x_ap[n][:, hs])
                xts.append(xt)

            # ---- constants -------------------------------------------------
            w2 = consts.tile([2 * K, D], f32)
            nc.sync.dma_start(out=w2[0:K, :], in_=w_ap)
            nc.sync.dma_start(out=w2[K : 2 * K, :], in_=w_ap)

            s_b = consts.tile([128, 1], f32)
            nc.gpsimd.dma_start(out=s_b, in_=s_ap.to_broadcast((128, 1)))
            inv_s2 = consts.tile([128, 1], f32)
            nc.vector.tensor_mul(inv_s2, s_b, s_b)
            nc.vector.reciprocal(inv_s2, inv_s2)

            ident = consts.tile([K, K], f32)
            make_identity(nc, ident)
            ps_w = pspool1.tile([D, K], f32)
            nc.tensor.transpose(ps_w, w2[0:K, :], ident)
            wT16 = consts.tile([D, K], f16)
            nc.vector.tensor_scalar_mul(wT16, in0=ps_w, scalar1=-2.0)

            ones16 = consts.tile([D, K], f16)
            nc.vector.memset(ones16, 1.0)

            w_sq = consts.tile([2 * K, D], f32)
            nc.vector.tensor_mul(w_sq, w2, w2)
            c_sq = consts.tile([2 * K, 1], f32)
            nc.vector.reduce_sum(out=c_sq, in_=w_sq, axis=mybir.AxisListType.X)

            # ---- derived stream: fp16 x^2 per n (from fp16 x) -------------
            xqs = []
            for n in range(NS):
                xq = xqpool.tile([D, L], f16, tag="xq", name=f"xsq_{n}")
                for h in range(2):
                    hs = slice(h * LH, (h + 1) * LH)
                    nc.scalar.activation(xq[:, hs], xts[n][:, hs], AF.Square)
                xqs.append(xq)

            # ---- matmuls + epilogue per pair ------------------------------
            spans = [(0, 1024), (1024, 2048), (2048, 3072), (3072, 3136)]
            out_ts = []
            for pair in range(NS // 2):
                n0, n1 = 2 * pair, 2 * pair + 1
                out_t = opool.tile([2 * K, L], f32, tag="out_t", name=f"out_{pair}")
                out_ts.append(out_t)
                for c0, c1 in spans:
                    ps = pspool.tile([2 * K, 1024], f32, name="ps")[:, : c1 - c0]
                    for b0 in range(0, c1 - c0, LC):
                        b1 = min(b0 + LC, c1 - c0)
                        pb = ps[:, b0:b1]
                        sl = slice(c0 + b0, c0 + b1)
                        nc.tensor.matmul(
                            pb[0:K, :], wT16, xts[n0][:, sl],
                            start=True, stop=False, tile_position=(0, 0),
                        )
                        nc.tensor.matmul(
                            pb[K : 2 * K, :], wT16, xts[n1][:, sl],
                            start=True, stop=False, tile_position=(0, 64),
                        )
                        nc.tensor.matmul(
                            pb[0:K, :], ones16, xqs[n0][:, sl],
                            start=False, stop=True, tile_position=(0, 0),
                        )
                        nc.tensor.matmul(
                            pb[K : 2 * K, :], ones16, xqs[n1][:, sl],
                            start=False, stop=True, tile_position=(0, 64),
                        )
                    nc.vector.tensor_scalar(
                        out=out_t[:, c0:c1], in0=ps,
                        scalar1=c_sq, scalar2=inv_s2,
                        op0=mybir.AluOpType.add, op1=mybir.AluOpType.mult,
                    )

            # ---- output stores on the scalar ring (tapered tail) ----------
            o0 = o_ap[0:2].rearrange("a k l -> (a k) l")
            nc.scalar.dma_start(out=o0, in_=out_ts[0])
            o1 = o_ap[2:4].rearrange("a k l -> (a k) l")
            for h in range(2):
                hs = slice(h * LH, (h + 1) * LH)
                nc.scalar.dma_start(out=o1[:, hs], in_=out_ts[1][:, hs])

    nc.compile()
    return nc


def _get_nc():
    if "nc" not in _cache:
        _cache["nc"] = _build()
    return _cache["nc"]


def run(x, weight, scale, trace=False):
    from concourse.bass_utils import run_bass_kernel_spmd

    x = np.ascontiguousarray(np.asarray(x, dtype=np.float32))
    weight = np.ascontiguousarray(np.asarray(weight, dtype=np.float32))
    scale = np.ascontiguousarray(np.asarray(scale, dtype=np.float32))
    assert x.shape == (N, D, L) and weight.shape == (K, D) and scale.shape == (1,)

    nc = _get_nc()
    in_maps = [
        {"x": x[c * NS : (c + 1) * NS], "weight": weight, "scale": scale}
        for c in range(N_CORES)
    ]
    res = run_bass_kernel_spmd(
        nc, in_maps, core_ids=list(range(N_CORES)), trace=trace
    )
    out = np.concatenate([r["out"] for r in res.results], axis=0)
    return out, res


def kernel(x, weight, scale):
    out, _ = run(x, weight, scale, trace=False)
    return out
